# revision 51
# baseline (speedup 1.0000x reference)
# Trainium2 Bass kernel for nn_AttnNCRFDecoder: multi-head attention + MLP
# head + 1-best Viterbi decode, data-parallel over batch across 8 NeuronCores.
#
# Device computes, per core (8 sequences): the forward model (attention,
# proj+residual+LayerNorm, MLP -> 33-tag logits, all fp32/fp32r matmuls) and
# the Viterbi value-DP over all 512 steps (delta history, on the Vector
# engine). The host does input transposition/sharding, weight folding, and
# the O(B*S*TAG) backtrace from the device-produced delta history (the
# backtrace argmaxes recompute exactly the device's fp32 sums, so decisions
# are bit-consistent with the device DP).
#
# Key layout for the DP: 128 partitions = 4 quadrants x 32 tags; quadrant q
# holds batch pair (2q, 2q+1) with b_lo selecting the pair member. Per step:
# two fused add+max-reduce ops (tensor_tensor_reduce), a logit add, then a
# 32x32 block transpose + two stream shuffles to rebroadcast the new delta
# (from-tag on the free axis, replicated across each quadrant's partitions).
import sys

for _p in ("/opt/trn_rl_repo",):
    if _p not in sys.path:
        sys.path.insert(0, _p)

import numpy as np

B, S, D = 64, 512, 768
H, DK, DV = 3, 64, 64
TAG, NBEST = 33, 8
START, STOP = TAG - 2, TAG - 1
NT = 32          # DP tag count (STOP dropped; START=31 present but dead)
NCORES = 8
BPC = B // NCORES  # sequences per core
DT = D // 128      # d-tiles
F32 = None         # set after mybir import

_BUILD_CACHE = {}


def _host_prep(inputs, w_qs, w_ks, w_vs, proj_w, proj_b, ln_g, ln_b,
               lin1_w, lin1_b, lin2_w, lin2_b, transitions):
    """Build the per-core input maps (all fp32, SBUF-layout-matched)."""
    f = np.float32
    x = np.ascontiguousarray(inputs, f)                       # (B,S,D)
    # xt[b, p, dt, s] = x[b, s, dt*128+p]
    xt = np.ascontiguousarray(
        x.reshape(B, S, DT, 128).transpose(0, 3, 2, 1), f)    # (B,128,6,512)

    wqk = np.zeros((128, H * DT * 128), f)
    for h in range(H):
        for dt in range(DT):
            blk = wqk[:, (h * DT + dt) * 128:(h * DT + dt + 1) * 128]
            blk[:, 0:64] = w_qs[h, dt * 128:(dt + 1) * 128, :]
            blk[:, 64:128] = w_ks[h, dt * 128:(dt + 1) * 128, :]
    wv01 = np.zeros((128, DT * 128), f)
    wv2 = np.zeros((128, DT * 64), f)
    for dt in range(DT):
        wv01[:, dt * 128:dt * 128 + 64] = w_vs[0, dt * 128:(dt + 1) * 128, :]
        wv01[:, dt * 128 + 64:(dt + 1) * 128] = w_vs[1, dt * 128:(dt + 1) * 128, :]
        wv2[:, dt * 64:(dt + 1) * 64] = w_vs[2, dt * 128:(dt + 1) * 128, :]

    wp_a = np.ascontiguousarray(proj_w[0:128, :], f)          # (128,768)
    wp_b = np.ascontiguousarray(proj_w[128:192, :], f)        # (64,768)
    projb = np.ascontiguousarray(proj_b.reshape(D // 128, 128).T, f)  # (128,6)

    w1eff = (ln_g[:, None] * lin1_w).astype(f)                # (768,384)
    w1s = np.zeros((128, DT * 384), f)
    for dt in range(DT):
        w1s[:, dt * 384:(dt + 1) * 384] = w1eff[dt * 128:(dt + 1) * 128, :]
    csw1 = np.ascontiguousarray(w1eff.sum(0).reshape(1, 384), f)
    b1eff = (ln_b @ lin1_w + lin1_b).astype(f)                # (384,)
    b1 = np.ascontiguousarray(b1eff.reshape(3, 128).T, f)     # (128,3)

    w2s = np.zeros((128, 3 * 128), f)
    for kt in range(3):
        blk = lin2_w[kt * 128:(kt + 1) * 128, 0:NT]
        w2s[:, kt * 128:(kt + 1) * 128] = np.tile(blk, (1, 4))
    l2b = np.tile(lin2_b[0:NT], 4).reshape(128, 1).astype(f)

    tr = transitions.astype(f)
    # transposed-DP layout: transT[32q+to, from] = trans[from, to]
    transT = np.ascontiguousarray(np.tile(tr[0:NT, 0:NT].T, (4, 1)), f)
    transstart = np.tile(tr[START, 0:NT], 4).reshape(128, 1).astype(f)

    ident = np.eye(128, dtype=f)
    ident64 = np.vstack([np.eye(64, dtype=f), np.eye(64, dtype=f)])  # (128,64)
    onesrow = np.ones((1, S), f)
    onescol = np.ones((128, 1), f)
    ones4 = np.ones((128, 4), f)

    shared = dict(wqk=wqk, wv01=wv01, wv2=wv2, wp_a=wp_a, wp_b=wp_b,
                  projb=projb, w1s=w1s, csw1=csw1, b1=b1, w2s=w2s, l2b=l2b,
                  transT=transT, transstart=transstart, ident=ident,
                  ident64=ident64, onesrow=onesrow, onescol=onescol,
                  ones4=ones4)
    in_maps = []
    for c in range(NCORES):
        m = dict(shared)
        m["xt"] = np.ascontiguousarray(xt[c * BPC:(c + 1) * BPC])
        in_maps.append(m)
    return in_maps


def build_nc():
    import concourse.bass as bass
    import concourse.mybir as mybir
    import concourse.tile as tile
    from concourse import bacc

    f32 = mybir.dt.float32
    f32r = mybir.dt.float32r
    ADD = mybir.AluOpType.add
    MULT = mybir.AluOpType.mult
    MAX = mybir.AluOpType.max
    AF = mybir.ActivationFunctionType

    nc = bacc.Bacc(None, target_bir_lowering=False, debug=False)

    def mm(out, lhsT, rhs, start, stop):
        nc.tensor.matmul(out, lhsT, rhs, start=start, stop=stop)

    with tile.TileContext(nc) as tc:
        from contextlib import ExitStack
        ctx = ExitStack()
        with ctx:
            dram = ctx.enter_context(tc.tile_pool(name="dram", bufs=1, space="DRAM"))

            def din(name, shape, dt_=None):
                return dram.tile(shape, dt_ or f32, kind="ExternalInput",
                                 name=name, uniquify=False)

            xt_d = din("xt", (BPC, 128, DT, S), f32r)
            wqk_d = din("wqk", (128, H * DT * 128), f32r)
            wv01_d = din("wv01", (128, DT * 128), f32r)
            wv2_d = din("wv2", (128, DT * 64), f32r)
            wpa_d = din("wp_a", (128, D), f32r)
            wpb_d = din("wp_b", (64, D), f32r)
            projb_d = din("projb", (128, DT))
            w1s_d = din("w1s", (128, DT * 384), f32r)
            csw1_d = din("csw1", (1, 384), f32r)
            b1_d = din("b1", (128, 3))
            w2s_d = din("w2s", (128, 3 * 128), f32r)
            l2b_d = din("l2b", (128, 1))
            transT_d = din("transT", (128, NT))
            transstart_d = din("transstart", (128, 1))
            ident_d = din("ident", (128, 128), f32r)
            ident64_d = din("ident64", (128, 64), f32r)
            onesrow_d = din("onesrow", (1, S), f32r)
            onescol_d = din("onescol", (128, 1), f32r)
            ones4_d = din("ones4", (128, 4), f32r)
            dh_d = dram.tile((128, 2 * S), f32, kind="ExternalOutput",
                             name="dh", uniquify=False)

            cp = ctx.enter_context(tc.tile_pool(name="consts", bufs=1))
            wqk_s = cp.tile((128, H * DT * 128), f32r)
            wv01_s = cp.tile((128, DT * 128), f32r)
            wv2_s = cp.tile((128, DT * 64), f32r)
            wpa_s = cp.tile((128, D), f32r)
            wpb_s = cp.tile((64, D), f32r)
            projb_s = cp.tile((128, DT), f32)
            w1s_s = cp.tile((128, DT * 384), f32r)
            csw1_s = cp.tile((1, 384), f32r)
            b1_s = cp.tile((128, 3), f32)
            w2s_s = cp.tile((128, 3 * 128), f32r)
            l2b_s = cp.tile((128, 1), f32)
            transT_s = cp.tile((128, NT), f32)
            transstart_s = cp.tile((128, 1), f32)
            ident_s = cp.tile((128, 128), f32r)
            ident64_s = cp.tile((128, 64), f32r)
            onesrow_s = cp.tile((1, S), f32r)
            onescol_s = cp.tile((128, 1), f32r)
            ones4_s = cp.tile((128, 4), f32r)

            # persistent DP state
            dpp = ctx.enter_context(tc.tile_pool(name="dp", bufs=1))
            logit_rep = dpp.tile((128, 2 * S), f32)   # [(q,to), (t,b_lo)]
            # hist[32q+to, 2t+b] = delta_t; +34 tail cols so the 32-wide
            # transpose read at t=S-1 stays in bounds
            hist = dpp.tile((128, 2 * S + 66), f32)
            dT0 = dpp.tile((128, NT), f32)    # per-quadrant transposed delta
            dT1 = dpp.tile((128, NT), f32)
            bc0 = dpp.tile((128, NT), f32)    # delta_{t-1}[b] bcast to 32 rows
            bc1 = dpp.tile((128, NT), f32)
            cand0 = dpp.tile((128, NT), f32)
            cand1 = dpp.tile((128, NT), f32)

            # rotating pools
            sb2 = ctx.enter_context(tc.tile_pool(name="sb2", bufs=2))
            sb3 = ctx.enter_context(tc.tile_pool(name="sb3", bufs=3))
            psB = ctx.enter_context(tc.tile_pool(name="psB", bufs=2, space="PSUM"))
            psS = ctx.enter_context(tc.tile_pool(name="psS", bufs=2, space="PSUM"))
            psV = ctx.enter_context(tc.tile_pool(name="psV", bufs=2, space="PSUM"))

            # seq-0 input first, then consts by first use, all issued from
            # gpsimd (25ns issue vs 565ns on SP) so PE can start QKV early
            xt_first = sb2.tile((128, DT * S), f32r, tag="xt", name="xt_first")
            nc.gpsimd.dma_start(out=xt_first[:], in_=xt_d[0])
            for sb, dr in [(wqk_s, wqk_d), (wv01_s, wv01_d), (wv2_s, wv2_d),
                           (ident64_s, ident64_d), (ones4_s, ones4_d),
                           (onescol_s, onescol_d), (onesrow_s, onesrow_d),
                           (wpa_s, wpa_d), (wpb_s, wpb_d), (projb_s, projb_d),
                           (w1s_s, w1s_d), (csw1_s, csw1_d), (b1_s, b1_d),
                           (w2s_s, w2s_d), (l2b_s, l2b_d),
                           (transT_s, transT_d), (transstart_s, transstart_d),
                           (ident_s, ident_d)]:
                nc.sync.dma_start(out=sb[:], in_=dr[:])

            import os as _os
            _skip_fwd = _os.environ.get("KSKIP_FWD") == "1"
            _skip_dp = _os.environ.get("KSKIP_DP") == "1"
            if _skip_fwd:
                nc.vector.memset(logit_rep[:], 0.0)
            for seq in ([] if _skip_fwd else range(BPC)):
                q32 = 32 * (seq // 2)
                b_lo = seq % 2

                if seq == 0:
                    xt_s = xt_first
                else:
                    xt_s = sb2.tile((128, DT * S), f32r, tag="xt")
                    nc.gpsimd.dma_start(out=xt_s[:], in_=xt_d[seq])

                # ---- QKV ----
                qT, kT = [], []
                for h in range(H):
                    pqk = psB.tile((128, S), f32, tag="big")
                    for dt in range(DT):
                        mm(pqk[:], wqk_s[:, (h * DT + dt) * 128:(h * DT + dt + 1) * 128],
                           xt_s[:, dt * S:(dt + 1) * S], dt == 0, dt == DT - 1)
                    q_sb = sb3.tile((64, S), f32r, tag="qT")
                    k_sb = sb3.tile((64, S), f32r, tag="kT")
                    nc.scalar.copy(q_sb[:], pqk[0:64, :])
                    nc.scalar.copy(k_sb[:], pqk[64:128, :])
                    qT.append(q_sb)
                    kT.append(k_sb)
                pv01 = psB.tile((128, S), f32, tag="big")
                for dt in range(DT):
                    mm(pv01[:], wv01_s[:, dt * 128:(dt + 1) * 128],
                       xt_s[:, dt * S:(dt + 1) * S], dt == 0, dt == DT - 1)
                vt01_sb = sb2.tile((128, S), f32r, tag="vt01")
                nc.scalar.copy(vt01_sb[:], pv01[:])
                pv2 = psB.tile((64, S), f32, tag="big")
                for dt in range(DT):
                    mm(pv2[:], wv2_s[:, dt * 64:(dt + 1) * 64],
                       xt_s[:, dt * S:(dt + 1) * S], dt == 0, dt == DT - 1)
                vt2_sb = sb2.tile((64, S), f32r, tag="vt2")
                nc.scalar.copy(vt2_sb[:], pv2[:])

                # V[j, v] per head via PE transpose of vT slices; col 64 of
                # each 65-wide block is ones so the AV matmul also produces
                # the softmax denominator (row 64 of the 65-row output)
                v_sb = []
                for h in range(H):
                    src = vt01_sb if h < 2 else vt2_sb
                    r0 = 64 * h if h < 2 else 0
                    vh = sb3.tile((128, 4 * 65), f32r, tag="v_sb")
                    vh_ones = bass.AP(
                        tensor=vh[:].tensor, offset=vh[:].offset + 64,
                        ap=[[vh[:].ap[0][0], 128], [65, 4]])
                    nc.vector.tensor_copy(vh_ones, ones4_s[:])
                    for st in range(4):
                        pvt = psV.tile((128, 64), f32r, tag="sm")
                        nc.tensor.transpose(
                            pvt[:], src[r0:r0 + 64, st * 128:(st + 1) * 128],
                            ident64_s[r0:r0 + 64, 0:64])
                        nc.vector.tensor_copy(vh[:, st * 65:st * 65 + 64], pvt[:])
                    v_sb.append(vh)

                # ---- attention per head ----
                oT_a = sb2.tile((128, S), f32r, tag="oT_a")   # heads 0,1
                oT_b = sb2.tile((64, S), f32r, tag="oT_b")    # head 2

                for h in range(H):
                    pT = sb2.tile((128, 4 * S), f32r, tag="pT")
                    for jt in range(4):
                        pst = psS.tile((128, S), f32, tag="st")
                        mm(pst[:], kT[h][:, jt * 128:(jt + 1) * 128],
                           qT[h][:], True, True)
                        # exp(s/8); also the PSUM->SBUF move
                        nc.scalar.activation(pT[:, jt * S:(jt + 1) * S], pst[:],
                                             AF.Exp, scale=0.125)
                    pav = psV.tile((65, S), f32, tag="sm")
                    for st in range(4):
                        mm(pav[:], v_sb[h][:, st * 65:(st + 1) * 65],
                           pT[:, st * S:(st + 1) * S], st == 0, st == 3)
                    rcp = sb2.tile((1, S), f32r, tag="rcp")
                    with nc.allow_low_precision("fp32r feed for matmul"):
                        nc.vector.reciprocal(rcp[:], pav[64:65, :])
                    prr = psV.tile((64, S), f32, tag="sm")
                    mm(prr[:], onesrow_s[:, 0:64], rcp[:], True, True)
                    rrep_sb = sb2.tile((64, S), f32, tag="rrep_sb")
                    nc.vector.tensor_copy(rrep_sb[:], prr[:])
                    odst = oT_a[64 * h:64 * (h + 1), :] if h < 2 else oT_b[:]
                    nc.vector.tensor_mul(odst, pav[0:64, :], rrep_sb[:])

                # ---- proj + residual + LN stats ----
                # z = proj(o) + projb + x fused: matmuls produce proj(o) in
                # PSUM; the PSUM->SBUF move is a scalar_tensor_tensor adding
                # projb (per-partition scalar) and the residual x
                z_sb = sb2.tile((128, DT * S), f32r, tag="z_sb")
                pmu = psS.tile((1, S), f32, tag="stat", bufs=2)
                ps2_ = psS.tile((1, S), f32, tag="stat", bufs=2)
                pmu = pmu[:]
                ps2_ = ps2_[:]
                for mt in range(DT):
                    pz = psB.tile((128, S), f32, tag="big")
                    mm(pz[:], wpa_s[:, mt * 128:(mt + 1) * 128], oT_a[:], True, False)
                    mm(pz[:], wpb_s[:, mt * 128:(mt + 1) * 128], oT_b[:], False, True)
                    with nc.allow_low_precision("fp32r z feed for matmul"):
                        nc.vector.scalar_tensor_tensor(
                            z_sb[:, mt * S:(mt + 1) * S], pz[:],
                            projb_s[:, mt:mt + 1], xt_s[:, mt * S:(mt + 1) * S],
                            ADD, ADD)
                    z2 = sb2.tile((128, S), f32r, tag="z2")
                    nc.gpsimd.tensor_mul(z2[:], z_sb[:, mt * S:(mt + 1) * S],
                                         z_sb[:, mt * S:(mt + 1) * S])
                    mm(pmu, onescol_s[:], z_sb[:, mt * S:(mt + 1) * S],
                       mt == 0, mt == DT - 1)
                    mm(ps2_, onescol_s[:], z2[:], mt == 0, mt == DT - 1)

                # ---- LN stats -> negmu, r ----
                negmu = sb2.tile((1, S), f32r, tag="stats1", bufs=8)
                nc.scalar.mul(negmu[:], pmu, -1.0 / D)
                mu2 = sb2.tile((1, S), f32, tag="stats1", bufs=8)
                nc.gpsimd.tensor_mul(mu2[:], negmu[:], negmu[:])
                varnum = sb2.tile((1, S), f32, tag="stats1", bufs=8)
                nc.vector.scalar_tensor_tensor(varnum[:], mu2[:], float(-D),
                                               ps2_, MULT, ADD)
                sig = sb2.tile((1, S), f32, tag="stats1", bufs=8)
                nc.scalar.activation(sig[:], varnum[:], AF.Sqrt,
                                     scale=1.0 / (D - 1))
                sige = sb2.tile((1, S), f32, tag="stats1", bufs=8)
                nc.vector.tensor_scalar_add(sige[:], sig[:], 1e-3)
                rln = sb2.tile((1, S), f32r, tag="stats1", bufs=8)
                with nc.allow_low_precision("fp32r feed for matmul"):
                    nc.vector.reciprocal(rln[:], sige[:])
                prl = psV.tile((128, S), f32, tag="sm")
                mm(prl[:], onesrow_s[:, 0:128], rln[:], True, True)
                rln_rep = sb2.tile((128, S), f32, tag="rln_rep")
                nc.scalar.copy(rln_rep[:], prl[:])

                # ---- lin1 (LN folded) + tanh ----
                hT = sb2.tile((128, 3 * S), f32r, tag="hT")
                for mt in range(3):
                    pg = psB.tile((128, S), f32, tag="big")
                    for kt in range(DT):
                        mm(pg[:], w1s_s[:, kt * 384 + mt * 128:kt * 384 + (mt + 1) * 128],
                           z_sb[:, kt * S:(kt + 1) * S], kt == 0, False)
                    mm(pg[:], csw1_s[:, mt * 128:(mt + 1) * 128], negmu[:],
                       False, True)
                    g_sb = sb2.tile((128, S), f32, tag="g_sb")
                    nc.vector.tensor_copy(g_sb[:], pg[:])
                    gr = sb2.tile((128, S), f32, tag="gr")
                    nc.gpsimd.tensor_mul(gr[:], g_sb[:], rln_rep[:])
                    nc.scalar.activation(hT[:, mt * S:(mt + 1) * S], gr[:],
                                         AF.Tanh, bias=b1_s[:, mt:mt + 1])

                # ---- lin2 -> logits into DP layout ----
                plg = psB.tile((128, S), f32, tag="big")
                for kt in range(3):
                    mm(plg[:], w2s_s[:, kt * 128:(kt + 1) * 128],
                       hT[:, kt * S:(kt + 1) * S], kt == 0, kt == 2)
                nc.scalar.activation(
                    logit_rep[q32:q32 + NT, b_lo::2], plg[q32:q32 + NT, :],
                    AF.Identity, bias=l2b_s[q32:q32 + NT, :])

            # ---- Viterbi value DP (transposed-broadcast scheme) ----
            # The two members of each quadrant's batch pair are fully
            # independent DPs; run them as two interleaved chains so each
            # chain's ~95ns dependent-issue gaps are hidden by the other's
            # ops. Per chain and step, on DVE:
            #   [T_b: 32x32 transpose of delta window at hist col t-1] ->
            #   [SS_b: broadcast quadrant row 0 (= delta_{t-1}) to 32 rows] ->
            #   [STT_b: cand = (bcast + logit_t[b]) + transT] ->
            #   [TR_b: hist_b[:, t] = max_from cand]
            # hist layout: b0 history at cols [0, S+32), b1 at [S+32, 2S+64).
            HB = S + 32
            nc.gpsimd.memset(hist[:], 0.0)
            if _skip_dp:
                nc.vector.tensor_copy(hist[:, 0:2 * S], logit_rep[:])
            S_dp = 1 if _skip_dp else S
            nc.vector.tensor_scalar_add(hist[:, 0:1], logit_rep[:, 0:1],
                                        transstart_s[:])
            nc.vector.tensor_scalar_add(hist[:, HB:HB + 1], logit_rep[:, 1:2],
                                        transstart_s[:])
            dTs = [dT0, dT1]
            bcs = [bc0, bc1]
            cands = [cand0, cand1]
            for t in range(1, S_dp):
                for b in (0, 1):
                    nc.vector.transpose(
                        dTs[b][:], hist[:, HB * b + t - 1:HB * b + t + 31])
                for b in (0, 1):
                    nc.vector.stream_shuffle(bcs[b][:], dTs[b][:], [0] * 32)
                for b in (0, 1):
                    nc.vector.scalar_tensor_tensor(
                        cands[b][:], bcs[b][:],
                        logit_rep[:, 2 * t + b:2 * t + b + 1],
                        transT_s[:], ADD, ADD)
                for b in (0, 1):
                    nc.vector.tensor_reduce(
                        hist[:, HB * b + t:HB * b + t + 1], cands[b][:],
                        mybir.AxisListType.X, MAX)

            nc.sync.dma_start(out=dh_d[:, 0:S], in_=hist[:, 0:S])
            nc.sync.dma_start(out=dh_d[:, S:2 * S], in_=hist[:, HB:HB + S])

    nc.compile()
    return nc


def _get_nc():
    if "nc" not in _BUILD_CACHE:
        _BUILD_CACHE["nc"] = build_nc()
    return _BUILD_CACHE["nc"]


def _backtrace(dh_all, lengths, transitions):
    """dh_all: (B, S, NT) device delta history. Vectorized over batch."""
    f = np.float32
    tr = transitions[0:NT, 0:NT].astype(f)           # [from, to]
    out = np.zeros((B, S), np.int32)
    # final tag per b at its own L-1
    last = dh_all[np.arange(B), lengths - 1, :] + transitions[0:NT, STOP][None, :]
    tag = last.argmax(1).astype(np.int64)            # (B,)
    out[:, S - 1] = tag
    out[np.arange(B), lengths - 1] = tag
    # walk all b in lockstep over t; only update b where t <= L-2
    cur = tag.copy()
    for t in range(S - 2, -1, -1):
        cand = dh_all[:, t, :] + tr[:, cur].T        # (B, NT) fp32
        prev = cand.argmax(1)
        active = t <= lengths - 2
        cur = np.where(active, prev, cur)
        out[:, t] = np.where(active, cur, out[:, t])
    return out


def kernel(**inputs):
    from concourse.bass_utils import run_bass_kernel_spmd

    args = {k: np.asarray(v) for k, v in inputs.items()}
    in_maps = _host_prep(
        args["inputs"], args["w_qs"], args["w_ks"], args["w_vs"],
        args["proj_w"], args["proj_b"], args["ln_g"], args["ln_b"],
        args["lin1_w"], args["lin1_b"], args["lin2_w"], args["lin2_b"],
        args["transitions"])

    nc = _get_nc()
    res = run_bass_kernel_spmd(nc, in_maps, core_ids=list(range(NCORES)))

    dh_all = np.zeros((B, S, NT), np.float32)
    for c in range(NCORES):
        dh = res.results[c]["dh"]                    # (128, 2S)
        # dh[32q+to, b*S+t] -> delta[b=2q+b_lo, t, to]
        d = dh.reshape(4, 32, 2, S).transpose(0, 2, 3, 1)  # (q, b_lo, t, to)
        dh_all[c * BPC:(c + 1) * BPC] = d.reshape(BPC, S, NT)

    lengths = np.asarray(args["labels_mask"]).astype(np.int64).sum(1)
    return _backtrace(dh_all, lengths, args["transitions"])

